# revision 58
# baseline (speedup 1.0000x reference)
"""Trainium2 Bass kernel for nn_Block_15066745274698 (GQA attention block).

Computation (B=1, T=4096, C=2048, 16 heads x 128, 4 KV groups):
  qkv = x @ W_attn.T ; split q/k/v ; RoPE(q, k) ; causal GQA attention ;
  out = y @ W_proj.T

Sharding: head-parallel over 8 cores, 2 query heads + their KV group per
core. No collectives: each core computes a partial out^T (its 2 heads
pushed through the matching W_proj columns); the host sums the 8 partials.

Device layout (per core) is transpose-oriented so every matmul contracts
over the partition dim with zero on-device transposes of activations:
  qkv^T (f x t) = W_attn_slice^T.T @ x^T      [via lhsT = W_attn^T tiles]
  S^T   (s x t) = K^T.T @ Q^T                 [scores transposed]
  y^T   (d x t) = V.T @ exp(S^T)              [V transposed once on PE]
  out^T (o x t) = W_proj_slice^T.T @ y^T

Softmax: no max-subtraction (scores bounded ~ +-5), exp on ACT with fused
1/sqrt(128) scale, causal handled by block skipping + gpsimd affine_select
on diagonal blocks, denominators via ones-vector matmul + DMA broadcast +
fast reciprocal.

Scheduling structure: score s-tiles are processed in PAIRS that share one
[128, 2, 512] PSUM slot (2 adjacent banks), so a single ACT instruction
computes exp over 1024 columns (halving ACT instruction overhead), and the
softmax denominator matmul runs on a DVE pair-sum of the two P tiles
(halving den PE rows + ones-LDWEIGHTS count). The same [128,2,512] slot
tag is reused for the QKV f-tile pairs and out-projection o-tile pairs so
PSUM is fully used (4 banks pairs + 1 transpose + 2 y + 1 den = 8) with a
2-deep rotation everywhere. Output partials are written bf16 (halves the
33MB outT stream; host accumulates in fp32), two o-tiles per descriptor.
"""
import sys

sys.path.insert(0, "/opt/trn_rl_repo")
import types

import numpy as np
import ml_dtypes

import os

import concourse.bass as bass
import concourse.mybir as mybir
import concourse.tile as tile
from concourse import bacc
from concourse.bass import ts
from concourse.bass_utils import run_bass_kernel_spmd
from concourse.masks import make_identity

if os.environ.get("K_LDWOPT", "0") == "1":
    import concourse.bass_utils as _bu

    _orig_run_command = _bu.run_command

    def _patched_run_command(argv, **kwargs):
        argv = [
            a.replace("--enable-ldw-opt=false", "--enable-ldw-opt=true")
            if isinstance(a, str)
            else a
            for a in argv
        ]
        return _orig_run_command(argv, **kwargs)

    _bu.run_command = _patched_run_command

T, C = 4096, 2048
HS = 128
TT = 512                 # t-tile (matmul moving free dim)
NT = T // TT             # 8
NCT = C // 128           # 16 c-tiles
F = 384                  # per-core W_attn rows: 2 q heads + (k XOR v) half
SCALE = 1.0 / float(np.sqrt(np.float32(HS)))

dt = mybir.dt
FP32 = dt.float32
BF16 = dt.bfloat16
AF = mybir.ActivationFunctionType
ALU = mybir.AluOpType

_cache = {}


def install_ntff_hook_shim():
    """antenv.axon_hooks is missing from this image; register the
    ctypes-based NTFF hook ourselves so trace=True works under axon."""
    if "antenv.axon_hooks" in sys.modules:
        return
    import antenv

    mod = types.ModuleType("antenv.axon_hooks")
    mod._hook = None
    mod.set_axon_ntff_profile_hook = lambda h: setattr(mod, "_hook", h)
    mod.get_axon_ntff_profile_hook = lambda: mod._hook
    sys.modules["antenv.axon_hooks"] = mod
    antenv.axon_hooks = mod
    try:
        from trn_agent_boot.trn_boot import _ntff_profile_via_ctypes

        mod.set_axon_ntff_profile_hook(
            _ntff_profile_via_ctypes("/opt/axon/libaxon_pjrt.so")
        )
    except Exception:
        pass


def _rope(nc, rtmp, cos_sl, sin_sl, qs, dst):
    """Rotate-half RoPE from SBUF bf16 staging qs (drained off the psum
    chain by one ACT copy, so the DVE never gates the psum slot). The sin
    table is stored partition-SWAPPED and sign-folded on the host
    (rows 0:64 = +sin, rows 64:128 = -sin), so both half-muls read their
    two SBUF operands at EQUAL base partitions:
      tsin[0:64]   = qs[64:128] * sin[64:128]   (= -x2*s)
      tsin[64:128] = qs[0:64]   * sin[0:64]     (= +x1*s)
      dst = qs * cos + tsin
    All operands 16-bit -> DVE 2x mode."""
    tcos = rtmp.tile([128, TT], BF16, tag="tcos")
    tsin = rtmp.tile([128, TT], BF16, tag="tsin")
    nc.vector.tensor_mul(tcos, qs, cos_sl)
    nc.vector.tensor_mul(tsin[0:64, :], qs[64:128, :], sin_sl[64:128, :])
    nc.vector.tensor_mul(tsin[64:128, :], qs[0:64, :], sin_sl[0:64, :])
    nc.vector.tensor_add(dst, tcos, tsin)


def build():
    nc = bacc.Bacc(
        "TRN2", target_bir_lowering=False, debug=False, enable_asserts=False,
        num_devices=8,
    )
    xT = nc.dram_tensor("xT", [C, T], BF16, kind="ExternalInput").ap()
    waT = nc.dram_tensor("waT", [C, F], BF16, kind="ExternalInput").ap()
    wpT = nc.dram_tensor("wpT", [2 * HS, C], BF16, kind="ExternalInput").ap()
    cos2 = nc.dram_tensor("cos2", [128, T], BF16, kind="ExternalInput").ap()
    sin2 = nc.dram_tensor("sin2", [128, T], BF16, kind="ExternalInput").ap()
    outT = nc.dram_tensor("outT", [C, T], BF16, kind="ExternalOutput").ap()
    # per-chunk k/v pair-exchange bounce buffers: each core computes only
    # its half (even cores k, odd cores v; same NEFF, different weights) and
    # the pair AllGathers [k; v] back. All QKV chunks run before any
    # attention so the serialized ~12-20us collectives are fully covered.
    kv_out = [
        nc.dram_tensor(f"kvo{i}", [128, TT], BF16).ap() for i in range(NT)
    ]
    kv_in = [
        nc.dram_tensor(f"kvi{i}", [256, TT], BF16).ap() for i in range(NT)
    ]
    CC_GROUPS = [[0, 1], [2, 3], [4, 5], [6, 7]]

    xT_r = xT.rearrange("(a p) t -> p a t", p=128)     # [128, 16, 4096]
    waT_r = waT.rearrange("(a p) f -> p a f", p=128)   # [128, 16, 512]
    wpT_r = wpT.rearrange("(a p) o -> p a o", p=128)   # [128, 2, 2048]
    outT_r = outT.rearrange("(a p) t -> p a t", p=128)  # [128, 16, 4096]

    with tile.TileContext(nc) as tc:
        with (
            tc.tile_pool(name="singles", bufs=1) as singles,
            tc.tile_pool(name="xp", bufs=4) as xp,
            tc.tile_pool(name="qp", bufs=2 * NT) as qp,
            tc.tile_pool(name="kp", bufs=NT) as kp,
            tc.tile_pool(name="vp", bufs=NT) as vp,
            tc.tile_pool(name="vstage", bufs=4) as vstage,
            tc.tile_pool(name="qstage", bufs=3) as qstage,
            tc.tile_pool(name="pp", bufs=6) as pp,
            tc.tile_pool(name="prp", bufs=3) as prp,
            tc.tile_pool(name="rtmp", bufs=4) as rtmp,
            tc.tile_pool(name="ysb", bufs=5) as ysb,
            tc.tile_pool(name="rbp", bufs=2) as rbp,
            tc.tile_pool(name="osb", bufs=4) as osb,
            tc.tile_pool(name="sp2", bufs=2, space="PSUM") as sp2,
            tc.tile_pool(name="y_ps", bufs=2, space="PSUM") as y_ps,
            tc.tile_pool(name="aux_ps", bufs=2, space="PSUM") as aux_ps,
        ):
            # ---- persistent tiles. DMA order matters: the very first qkv
            # matmuls need wa chunk 0 + x chunk 0; cos/sin slice 0 follows
            # immediately (rope for chunk 0 needs it ~10us in); bulk after.
            wa_sb = singles.tile([128, NCT, F], BF16)
            xt0 = xp.tile([128, NCT, TT], BF16, tag="xt")
            cos_sb = singles.tile([128, T], BF16)
            sin_sb = singles.tile([128, T], BF16)
            nc.sync.dma_start(wa_sb[:, 0:1, :], waT_r[:, 0:1, :])
            nc.scalar.dma_start(xt0[:, 0:1, :], xT_r[:, 0:1, 0:TT])
            nc.gpsimd.dma_start(cos_sb[:, 0:TT], cos2[:, 0:TT])
            nc.gpsimd.dma_start(sin_sb[:, 0:TT], sin2[:, 0:TT])
            nc.sync.dma_start(wa_sb[:, 1:2, :], waT_r[:, 1:2, :])
            nc.scalar.dma_start(xt0[:, 1:2, :], xT_r[:, 1:2, 0:TT])
            nc.sync.dma_start(wa_sb[:, 2:4, :], waT_r[:, 2:4, :])
            nc.scalar.dma_start(xt0[:, 2:4, :], xT_r[:, 2:4, 0:TT])
            # xt for prologue chunks 1 and 2 is prefetched here, interleaved
            # with xt0's and wa's bulk, so the qkv(1)/qkv(2) chains never
            # wait on the x stream (was an ~12us PE stall)
            xt1 = xp.tile([128, NCT, TT], BF16, tag="xt")
            xt2 = xp.tile([128, NCT, TT], BF16, tag="xt")
            for q in range(1, 4):
                nc.sync.dma_start(
                    wa_sb[:, 4 * q:4 * (q + 1), :], waT_r[:, 4 * q:4 * (q + 1), :]
                )
                nc.scalar.dma_start(
                    xt0[:, 4 * q:4 * (q + 1), :],
                    xT_r[:, 4 * q:4 * (q + 1), 0:TT],
                )
                nc.scalar.dma_start(
                    xt1[:, 4 * (q - 1):4 * q, :],
                    xT_r[:, 4 * (q - 1):4 * q, TT:2 * TT],
                )
            nc.scalar.dma_start(xt1[:, 12:16, :], xT_r[:, 12:16, TT:2 * TT])
            for q in range(4):
                nc.sync.dma_start(
                    xt2[:, 4 * q:4 * (q + 1), :],
                    xT_r[:, 4 * q:4 * (q + 1), 2 * TT:3 * TT],
                )
            nc.gpsimd.dma_start(cos_sb[:, TT:], cos2[:, TT:])
            nc.gpsimd.dma_start(sin_sb[:, TT:], sin2[:, TT:])
            wp_sb = singles.tile([128, 2, C], BF16)
            nc.gpsimd.dma_start(wp_sb, wpT_r)
            ident = singles.tile([128, 128], BF16)
            make_identity(nc, ident)
            ones_sb = singles.tile([128, 1], BF16)
            nc.vector.memset(ones_sb, 1.0)
            F32R = dt.float32r
            ones_colf = singles.tile([1, 128], FP32)
            nc.vector.memset(ones_colf, 1.0)
            ones_col = singles.tile([1, 128], F32R)
            nc.vector.tensor_copy(ones_col, ones_colf)

            q_tiles = [[None] * NT for _ in range(2)]
            k_tiles = [None] * NT
            v_tiles = [None] * NT
            y_chunks = [[] for _ in range(NT)]

            def proj_pair(i, op2):
                # one pair of out-projection o-tiles for t-chunk i: a psum
                # pair-slot, drains split vector/scalar, one store
                # descriptor (bf16). Interleaved into the NEXT chunk's
                # attention pair loop so attention matmuls cover the drain
                # latency.
                slot = sp2.tile([128, 2, TT], FP32, tag="s2")
                ost = osb.tile([128, 2, TT], BF16, tag="ot")
                for h2 in range(2):
                    oi = 2 * op2 + h2
                    for cj in range(2):
                        nc.tensor.matmul(
                            slot[:, h2, :],
                            wp_sb[:, cj, oi * 128:(oi + 1) * 128],
                            y_chunks[i][cj],
                            start=(cj == 0),
                            stop=(cj == 1),
                        )
                    if h2 == 0:
                        nc.vector.tensor_copy(ost[:, 0, :], slot[:, 0, :])
                    else:
                        nc.scalar.copy(ost[:, 1, :], slot[:, 1, :])
                eng = nc.sync if op2 % 2 == 0 else nc.gpsimd
                eng.dma_start(outT_r[:, 2 * op2:2 * op2 + 2, ts(i, TT)], ost)

            def emit_proj(i):
                for op2 in range(NCT // 2):
                    proj_pair(i, op2)

            def qkv(ii):
                # QKV chains for chunk ii; the kv-half chain runs FIRST so
                # its pair AllGather is issued as early as possible
                if ii <= 2:
                    xt = (xt0, xt1, xt2)[ii]
                else:
                    xt = xp.tile([128, NCT, TT], BF16, tag="xt")
                    eng = (nc.scalar, nc.sync)[ii % 2]
                    for q in range(4):
                        eng.dma_start(
                            xt[:, 4 * q:4 * (q + 1), :],
                            xT_r[:, 4 * q:4 * (q + 1), ts(ii, TT)],
                        )

                def qkv_chain(slot_half, f):
                    for ci in range(NCT):
                        nc.tensor.matmul(
                            slot_half,
                            wa_sb[:, ci, f * 128:(f + 1) * 128],
                            xt[:, ci, :],
                            start=(ci == 0),
                            stop=(ci == NCT - 1),
                        )

                def qkv_rope(slot_half, f):
                    # drain the chain via one ACT copy so the psum slot is
                    # freed without touching the DVE queue
                    qs = qstage.tile([128, TT], BF16, tag="qs")
                    nc.scalar.copy(qs, slot_half)
                    dst = qp.tile([128, TT], BF16, tag="qt")
                    q_tiles[f][ii] = dst
                    _rope(nc, rtmp, cos_sb[:, ts(ii, TT)],
                          sin_sb[:, ts(ii, TT)], qs, dst)

                slotA = sp2.tile([128, 2, TT], FP32, tag="s2")
                qkv_chain(slotA[:, 0, :], 2)
                kvs = vstage.tile([128, TT], BF16, tag="vst")
                nc.scalar.copy(kvs, slotA[:, 0, :])
                nc.sync.dma_start(kv_out[ii], kvs)
                nc.gpsimd.collective_compute(
                    "AllGather",
                    ALU.bypass,
                    replica_groups=CC_GROUPS,
                    ins=[kv_out[ii]],
                    outs=[kv_in[ii]],
                )
                qkv_chain(slotA[:, 1, :], 0)
                qkv_rope(slotA[:, 1, :], 0)
                slotB = sp2.tile([128, 2, TT], FP32, tag="s2")
                qkv_chain(slotB[:, 0, :], 1)
                qkv_rope(slotB[:, 0, :], 1)

            def receive(ii):
                # land the pair-exchanged [k; v] for chunk ii, rope k, and
                # build V via PE transposes. DMAs go on the sync queue so
                # they never queue behind later collectives on gpsimd.
                kvr = kv_in[ii].rearrange("(a p) t -> p a t", p=128)
                kraw = qstage.tile([128, TT], BF16, tag="qs")
                vst = vstage.tile([128, TT], BF16, tag="vst")
                nc.sync.dma_start(kraw, kvr[:, 0, :])
                nc.sync.dma_start(vst, kvr[:, 1, :])
                dstk = kp.tile([128, TT], BF16, tag="kt")
                k_tiles[ii] = dstk
                _rope(nc, rtmp, cos_sb[:, ts(ii, TT)],
                      sin_sb[:, ts(ii, TT)], kraw, dstk)
                v4 = vp.tile([128, 4, 128], BF16, tag="vt")
                v_tiles[ii] = v4
                for j4 in range(4):
                    tp = y_ps.tile([128, 128], BF16, tag="y")
                    nc.tensor.transpose(
                        tp, vst[:, j4 * 128:(j4 + 1) * 128], ident
                    )
                    nc.vector.tensor_copy(v4[:, j4, :], tp)

            # QKV runs three chunks ahead of attention: ~30us of PE work in
            # flight always covers the serialized ~12-20us collectives, and
            # x-chunk DMA demand stays spread over the whole kernel
            for ii in range(3):
                qkv(ii)

            for i in range(NT):
                if i + 3 < NT:
                    qkv(i + 3)
                receive(i)

                # ---- attention for t-chunk i, both heads ----
                # the previous chunk's out-projection pairs are interleaved
                # into this chunk's attention pair stream (one proj pair per
                # flushed attention pair) so attention matmuls fill the
                # proj-drain latency
                proj_queue = (
                    [(i - 1, op2) for op2 in range(NCT // 2)] if i > 0 else []
                )
                yts = y_chunks[i]
                ns = 4 * (i + 1)
                for h in range(2):
                    yp = y_ps.tile([128, TT], FP32, tag="y")
                    dp = aux_ps.tile([1, TT], FP32, tag="den")
                    den_started = [False]

                    def den_mm(ap, off, is_last):
                        nc.tensor.matmul(
                            dp[:, off:], ones_sb, ap,
                            start=(not den_started[0]), stop=is_last,
                            skip_group_check=True,
                        )
                        den_started[0] = True

                    def flush(m, offs, p2, prd, diag):
                        # AV + den for pair m; emitted one pair late so the
                        # ACT exp latency is hidden behind the next pair's
                        # score matmuls
                        j0 = 2 * m
                        for h2 in range(2):
                            j = j0 + h2
                            off = offs[h2]
                            nc.tensor.matmul(
                                yp[:, off:], v_tiles[j // 4][:, j % 4, :],
                                p2[:, h2, off:],
                                start=(j == 0), stop=(j == ns - 1),
                                skip_group_check=True,
                            )
                        if prd is not None:
                            den_mm(prd, 0, False)
                        else:
                            for h2 in range(2):
                                j = j0 + h2
                                den_mm(p2[:, h2, offs[h2]:], offs[h2],
                                       j == ns - 1)

                    pend = []
                    for m in range(ns // 2):
                        j0 = 2 * m
                        diag = j0 >= 4 * i
                        s2t = sp2.tile([128, 2, TT], FP32, tag="s2")
                        p2 = pp.tile([128, 2, TT], BF16, tag="p")
                        offs = []
                        for h2 in range(2):
                            j = j0 + h2
                            off = (j % 4) * 128 if j >= 4 * i else 0
                            offs.append(off)
                            nc.tensor.matmul(
                                s2t[:, h2, off:],
                                k_tiles[j // 4][:, (j % 4) * 128:(j % 4 + 1) * 128],
                                q_tiles[h][i][:, off:],
                                start=True,
                                stop=True,
                            )
                        prd = None
                        if not diag:
                            # one ACT instruction for both halves (1024 cols)
                            nc.scalar.activation(p2, s2t, AF.Exp, scale=SCALE)
                            prd = prp.tile([128, TT], BF16, tag="pr")
                            nc.vector.tensor_add(prd, p2[:, 0, :], p2[:, 1, :])
                        else:
                            for h2 in range(2):
                                j = j0 + h2
                                off = offs[h2]
                                nv = TT - off
                                nc.scalar.activation(
                                    p2[:, h2, off:], s2t[:, h2, off:],
                                    AF.Exp, scale=SCALE,
                                )
                                # zero entries with s > t inside the aligned
                                # 128-wide triangle at the start of the
                                # slice: keep iff y - p >= 0
                                nc.gpsimd.affine_select(
                                    out=p2[:, h2, off:],
                                    in_=p2[:, h2, off:],
                                    compare_op=ALU.is_ge,
                                    fill=0.0,
                                    base=0,
                                    pattern=[[1, nv]],
                                    channel_multiplier=-1,
                                )
                        pend.append((m, offs, p2, prd, diag))
                        if len(pend) > 2:
                            flush(*pend.pop(0))
                            if proj_queue:
                                proj_pair(*proj_queue.pop(0))
                    for pe in pend:
                        flush(*pe)
                        if proj_queue:
                            proj_pair(*proj_queue.pop(0))

                    # softmax denominator -> broadcast to 128 rows via a tiny
                    # K=1 fp32r matmul on PE, then fast reciprocal
                    drow = rbp.tile([1, TT], F32R, tag="drow")
                    nc.vector.tensor_copy(drow, dp)
                    bc = aux_ps.tile([128, TT], FP32, tag="den")
                    nc.tensor.matmul(bc, ones_col, drow, start=True, stop=True)
                    rb = rbp.tile([128, TT], FP32, tag="rb")
                    nc.vector.reciprocal_approx_fast(out=rb, in_=bc)
                    yt = ysb.tile([128, TT], BF16, tag="yt")
                    nc.vector.tensor_mul(yt, yp, rb)
                    yts.append(yt)

                for pq in proj_queue:
                    proj_pair(*pq)

            emit_proj(NT - 1)

    nc.compile()
    return nc


def _prep_inputs(x, cos, sin, W_attn, W_proj):
    bf = ml_dtypes.bfloat16
    x = np.asarray(x, dtype=np.float32)
    cos = np.asarray(cos, dtype=np.float32)
    sin = np.asarray(sin, dtype=np.float32)
    W_attn = np.asarray(W_attn, dtype=np.float32)
    W_proj = np.asarray(W_proj, dtype=np.float32)

    xT = np.ascontiguousarray(x.reshape(T, C).T).astype(bf)
    cos2 = np.ascontiguousarray(np.concatenate([cos.T, cos.T], axis=0)).astype(bf)
    # partition-swapped + sign-folded: rows 0:64 = +sin (multiplies x1 into
    # the upper output half), rows 64:128 = -sin (multiplies x2 into the
    # lower half); see _rope
    sin2 = np.ascontiguousarray(
        np.concatenate([sin.T, -sin.T], axis=0)
    ).astype(bf)

    in_maps = []
    for core in range(8):
        g = core // 2
        qoff = g * 768 + (core % 2) * 256
        # even core of a pair computes the group's K rows, odd computes V;
        # the pair AllGathers so both end up with [k; v]
        kv = (
            W_attn[g * 768 + 512:g * 768 + 640]
            if core % 2 == 0
            else W_attn[g * 768 + 640:g * 768 + 768]
        )
        rows = np.concatenate([W_attn[qoff:qoff + 256], kv], axis=0)
        waT = np.ascontiguousarray(rows.T).astype(bf)
        h0 = g * 4 + (core % 2) * 2
        wpT = np.ascontiguousarray(W_proj[:, h0 * 128:h0 * 128 + 256].T).astype(bf)
        in_maps.append(
            {"xT": xT, "waT": waT, "wpT": wpT, "cos2": cos2, "sin2": sin2}
        )
    return in_maps


def kernel(x, cos, sin, W_attn, W_proj, _trace=False, _trace_cores=None):
    if "nc" not in _cache:
        _cache["nc"] = build()
    nc = _cache["nc"]
    in_maps = _prep_inputs(x, cos, sin, W_attn, W_proj)
    kwargs = {}
    if _trace:
        install_ntff_hook_shim()
        kwargs = dict(trace=True, trace_cores=_trace_cores or [0])
    res = run_bass_kernel_spmd(nc, in_maps, core_ids=list(range(8)), **kwargs)
    acc = np.zeros((C, T), dtype=np.float32)
    for r in res.results:
        acc += np.asarray(r["outT"], dtype=np.float32)
    out = np.ascontiguousarray(acc.T).reshape(1, T, C)
    _cache["last_results"] = res
    return out


# revision 64
# speedup vs baseline: 1.0550x; 1.0550x over previous
"""Trainium2 Bass kernel for nn_Block_15066745274698 (GQA attention block).

Computation (B=1, T=4096, C=2048, 16 heads x 128, 4 KV groups):
  qkv = x @ W_attn.T ; split q/k/v ; RoPE(q, k) ; causal GQA attention ;
  out = y @ W_proj.T

Sharding: head-parallel over 8 cores, 2 query heads + their KV group per
core. No collectives: each core computes a partial out^T (its 2 heads
pushed through the matching W_proj columns); the host sums the 8 partials.

Device layout (per core) is transpose-oriented so every matmul contracts
over the partition dim with zero on-device transposes of activations:
  qkv^T (f x t) = W_attn_slice^T.T @ x^T      [via lhsT = W_attn^T tiles]
  S^T   (s x t) = K^T.T @ Q^T                 [scores transposed]
  y^T   (d x t) = V.T @ exp(S^T)              [V transposed once on PE]
  out^T (o x t) = W_proj_slice^T.T @ y^T

Softmax: no max-subtraction (scores bounded ~ +-5), exp on ACT with fused
1/sqrt(128) scale, causal handled by block skipping + gpsimd affine_select
on diagonal blocks, denominators via ones-vector matmul + DMA broadcast +
fast reciprocal.

Scheduling structure: score s-tiles are processed in PAIRS that share one
[128, 2, 512] PSUM slot (2 adjacent banks), so a single ACT instruction
computes exp over 1024 columns (halving ACT instruction overhead), and the
softmax denominator matmul runs on a DVE pair-sum of the two P tiles
(halving den PE rows + ones-LDWEIGHTS count). The same [128,2,512] slot
tag is reused for the QKV f-tile pairs and out-projection o-tile pairs so
PSUM is fully used (4 banks pairs + 1 transpose + 2 y + 1 den = 8) with a
2-deep rotation everywhere. Output partials are written bf16 (halves the
33MB outT stream; host accumulates in fp32), two o-tiles per descriptor.
"""
import sys

sys.path.insert(0, "/opt/trn_rl_repo")
import types

import numpy as np
import ml_dtypes

import os

import concourse.bass as bass
import concourse.mybir as mybir
import concourse.tile as tile
from concourse import bacc
from concourse.bass import ts
from concourse.bass_utils import run_bass_kernel_spmd
from concourse.masks import make_identity

if os.environ.get("K_LDWOPT", "0") == "1":
    import concourse.bass_utils as _bu

    _orig_run_command = _bu.run_command

    def _patched_run_command(argv, **kwargs):
        argv = [
            a.replace("--enable-ldw-opt=false", "--enable-ldw-opt=true")
            if isinstance(a, str)
            else a
            for a in argv
        ]
        return _orig_run_command(argv, **kwargs)

    _bu.run_command = _patched_run_command

T, C = 4096, 2048
HS = 128
TT = 512                 # t-tile (matmul moving free dim)
NT = T // TT             # 8
NCT = C // 128           # 16 c-tiles
F = 384                  # per-core W_attn rows: 2 q heads + (k XOR v) half
SCALE = 1.0 / float(np.sqrt(np.float32(HS)))

dt = mybir.dt
FP32 = dt.float32
BF16 = dt.bfloat16
AF = mybir.ActivationFunctionType
ALU = mybir.AluOpType

_cache = {}


def install_ntff_hook_shim():
    """antenv.axon_hooks is missing from this image; register the
    ctypes-based NTFF hook ourselves so trace=True works under axon."""
    if "antenv.axon_hooks" in sys.modules:
        return
    import antenv

    mod = types.ModuleType("antenv.axon_hooks")
    mod._hook = None
    mod.set_axon_ntff_profile_hook = lambda h: setattr(mod, "_hook", h)
    mod.get_axon_ntff_profile_hook = lambda: mod._hook
    sys.modules["antenv.axon_hooks"] = mod
    antenv.axon_hooks = mod
    try:
        from trn_agent_boot.trn_boot import _ntff_profile_via_ctypes

        mod.set_axon_ntff_profile_hook(
            _ntff_profile_via_ctypes("/opt/axon/libaxon_pjrt.so")
        )
    except Exception:
        pass


def _rope(nc, rtmp, cos_sl, sin_sl, qs, dst):
    """Rotate-half RoPE from SBUF bf16 staging qs (drained off the psum
    chain by one ACT copy, so the DVE never gates the psum slot). The sin
    table is stored partition-SWAPPED and sign-folded on the host
    (rows 0:64 = +sin, rows 64:128 = -sin), so both half-muls read their
    two SBUF operands at EQUAL base partitions:
      tsin[0:64]   = qs[64:128] * sin[64:128]   (= -x2*s)
      tsin[64:128] = qs[0:64]   * sin[0:64]     (= +x1*s)
      dst = qs * cos + tsin
    All operands 16-bit -> DVE 2x mode."""
    tcos = rtmp.tile([128, TT], BF16, tag="tcos")
    tsin = rtmp.tile([128, TT], BF16, tag="tsin")
    nc.vector.tensor_mul(tcos, qs, cos_sl)
    nc.vector.tensor_mul(tsin[0:64, :], qs[64:128, :], sin_sl[64:128, :])
    nc.vector.tensor_mul(tsin[64:128, :], qs[0:64, :], sin_sl[0:64, :])
    nc.vector.tensor_add(dst, tcos, tsin)


def build():
    nc = bacc.Bacc(
        "TRN2", target_bir_lowering=False, debug=False, enable_asserts=False,
        num_devices=8,
    )
    xT = nc.dram_tensor("xT", [C, T], BF16, kind="ExternalInput").ap()
    waT = nc.dram_tensor("waT", [C, F], BF16, kind="ExternalInput").ap()
    wpT = nc.dram_tensor("wpT", [2 * HS, C], BF16, kind="ExternalInput").ap()
    cos2 = nc.dram_tensor("cos2", [128, T], BF16, kind="ExternalInput").ap()
    sin2 = nc.dram_tensor("sin2", [128, T], BF16, kind="ExternalInput").ap()
    outT = nc.dram_tensor("outT", [C, T], BF16, kind="ExternalOutput").ap()
    # per-chunk k/v pair-exchange bounce buffers: each core computes only
    # its half (even cores k, odd cores v; same NEFF, different weights) and
    # the pair AllGathers [k; v] back. All QKV chunks run before any
    # attention so the serialized ~12-20us collectives are fully covered.
    kv_out = [
        nc.dram_tensor(f"kvo{i}", [128, TT], BF16).ap() for i in range(NT)
    ]
    kv_in = [
        nc.dram_tensor(f"kvi{i}", [256, TT], BF16).ap() for i in range(NT)
    ]
    CC_GROUPS = [[0, 1], [2, 3], [4, 5], [6, 7]]

    xT_r = xT.rearrange("(a p) t -> p a t", p=128)     # [128, 16, 4096]
    waT_r = waT.rearrange("(a p) f -> p a f", p=128)   # [128, 16, 512]
    wpT_r = wpT.rearrange("(a p) o -> p a o", p=128)   # [128, 2, 2048]
    outT_r = outT.rearrange("(a p) t -> p a t", p=128)  # [128, 16, 4096]

    with tile.TileContext(nc) as tc:
        with (
            tc.tile_pool(name="singles", bufs=1) as singles,
            tc.tile_pool(name="xp", bufs=4) as xp,
            tc.tile_pool(name="qp", bufs=2 * NT) as qp,
            tc.tile_pool(name="kp", bufs=NT) as kp,
            tc.tile_pool(name="vp", bufs=NT) as vp,
            tc.tile_pool(name="vstage", bufs=4) as vstage,
            tc.tile_pool(name="qstage", bufs=3) as qstage,
            tc.tile_pool(name="pp", bufs=6) as pp,
            tc.tile_pool(name="prp", bufs=5) as prp,
            tc.tile_pool(name="rtmp", bufs=4) as rtmp,
            tc.tile_pool(name="ysb", bufs=5) as ysb,
            tc.tile_pool(name="rbp", bufs=2) as rbp,
            tc.tile_pool(name="osb", bufs=4) as osb,
            tc.tile_pool(name="sp2", bufs=2, space="PSUM") as sp2,
            tc.tile_pool(name="y_ps", bufs=2, space="PSUM") as y_ps,
            tc.tile_pool(name="aux_ps", bufs=2, space="PSUM") as aux_ps,
        ):
            # ---- persistent tiles. DMA order matters: the very first qkv
            # matmuls need wa chunk 0 + x chunk 0; cos/sin slice 0 follows
            # immediately (rope for chunk 0 needs it ~10us in); bulk after.
            wa_sb = singles.tile([128, NCT, F], BF16)
            xt0 = xp.tile([128, NCT, TT], BF16, tag="xt")
            cos_sb = singles.tile([128, T], BF16)
            sin_sb = singles.tile([128, T], BF16)
            nc.sync.dma_start(wa_sb[:, 0:1, :], waT_r[:, 0:1, :])
            nc.scalar.dma_start(xt0[:, 0:1, :], xT_r[:, 0:1, 0:TT])
            nc.gpsimd.dma_start(cos_sb[:, 0:TT], cos2[:, 0:TT])
            nc.gpsimd.dma_start(sin_sb[:, 0:TT], sin2[:, 0:TT])
            nc.sync.dma_start(wa_sb[:, 1:2, :], waT_r[:, 1:2, :])
            nc.scalar.dma_start(xt0[:, 1:2, :], xT_r[:, 1:2, 0:TT])
            nc.sync.dma_start(wa_sb[:, 2:4, :], waT_r[:, 2:4, :])
            nc.scalar.dma_start(xt0[:, 2:4, :], xT_r[:, 2:4, 0:TT])
            for q in range(1, 4):
                nc.sync.dma_start(
                    wa_sb[:, 4 * q:4 * (q + 1), :], waT_r[:, 4 * q:4 * (q + 1), :]
                )
                nc.scalar.dma_start(
                    xt0[:, 4 * q:4 * (q + 1), :],
                    xT_r[:, 4 * q:4 * (q + 1), 0:TT],
                )
            nc.gpsimd.dma_start(cos_sb[:, TT:], cos2[:, TT:])
            nc.gpsimd.dma_start(sin_sb[:, TT:], sin2[:, TT:])
            wp_sb = singles.tile([128, 2, C], BF16)
            nc.gpsimd.dma_start(wp_sb, wpT_r)
            ident = singles.tile([128, 128], BF16)
            make_identity(nc, ident)
            ones_sb = singles.tile([128, 1], BF16)
            nc.vector.memset(ones_sb, 1.0)
            F32R = dt.float32r
            ones_colf = singles.tile([1, 128], FP32)
            nc.vector.memset(ones_colf, 1.0)
            ones_col = singles.tile([1, 128], F32R)
            nc.vector.tensor_copy(ones_col, ones_colf)

            q_tiles = [[None] * NT for _ in range(2)]
            k_tiles = [None] * NT
            v_tiles = [None] * NT
            y_chunks = [[] for _ in range(NT)]

            def proj_pair(i, op2):
                # one pair of out-projection o-tiles for t-chunk i: a psum
                # pair-slot, drains split vector/scalar, one store
                # descriptor (bf16). Interleaved into the NEXT chunk's
                # attention pair loop so attention matmuls cover the drain
                # latency.
                slot = sp2.tile([128, 2, TT], FP32, tag="s2")
                ost = osb.tile([128, 2, TT], BF16, tag="ot")
                for h2 in range(2):
                    oi = 2 * op2 + h2
                    for cj in range(2):
                        nc.tensor.matmul(
                            slot[:, h2, :],
                            wp_sb[:, cj, oi * 128:(oi + 1) * 128],
                            y_chunks[i][cj],
                            start=(cj == 0),
                            stop=(cj == 1),
                        )
                    if h2 == 0:
                        nc.vector.tensor_copy(ost[:, 0, :], slot[:, 0, :])
                    else:
                        nc.scalar.copy(ost[:, 1, :], slot[:, 1, :])
                eng = nc.sync if op2 % 2 == 0 else nc.gpsimd
                eng.dma_start(outT_r[:, 2 * op2:2 * op2 + 2, ts(i, TT)], ost)

            def emit_proj(i):
                for op2 in range(NCT // 2):
                    proj_pair(i, op2)

            def qkv(ii):
                # QKV chains for chunk ii; the kv-half chain runs FIRST so
                # its pair AllGather is issued as early as possible
                if ii == 0:
                    xt = xt0
                else:
                    xt = xp.tile([128, NCT, TT], BF16, tag="xt")
                    eng = (nc.scalar, nc.sync)[ii % 2]
                    for q in range(4):
                        eng.dma_start(
                            xt[:, 4 * q:4 * (q + 1), :],
                            xT_r[:, 4 * q:4 * (q + 1), ts(ii, TT)],
                        )

                def qkv_chain(slot_half, f):
                    for ci in range(NCT):
                        nc.tensor.matmul(
                            slot_half,
                            wa_sb[:, ci, f * 128:(f + 1) * 128],
                            xt[:, ci, :],
                            start=(ci == 0),
                            stop=(ci == NCT - 1),
                        )

                def qkv_rope(slot_half, f):
                    # drain the chain via one ACT copy so the psum slot is
                    # freed without touching the DVE queue
                    qs = qstage.tile([128, TT], BF16, tag="qs")
                    nc.scalar.copy(qs, slot_half)
                    dst = qp.tile([128, TT], BF16, tag="qt")
                    q_tiles[f][ii] = dst
                    _rope(nc, rtmp, cos_sb[:, ts(ii, TT)],
                          sin_sb[:, ts(ii, TT)], qs, dst)

                slotA = sp2.tile([128, 2, TT], FP32, tag="s2")
                qkv_chain(slotA[:, 0, :], 2)
                kvs = vstage.tile([128, TT], BF16, tag="vst")
                nc.scalar.copy(kvs, slotA[:, 0, :])
                nc.sync.dma_start(kv_out[ii], kvs)
                nc.gpsimd.collective_compute(
                    "AllGather",
                    ALU.bypass,
                    replica_groups=CC_GROUPS,
                    ins=[kv_out[ii]],
                    outs=[kv_in[ii]],
                )
                qkv_chain(slotA[:, 1, :], 0)
                qkv_rope(slotA[:, 1, :], 0)
                slotB = sp2.tile([128, 2, TT], FP32, tag="s2")
                qkv_chain(slotB[:, 0, :], 1)
                qkv_rope(slotB[:, 0, :], 1)

            def receive(ii):
                # land the pair-exchanged [k; v] for chunk ii, rope k, and
                # build V via PE transposes. DMAs go on the sync queue so
                # they never queue behind later collectives on gpsimd.
                kvr = kv_in[ii].rearrange("(a p) t -> p a t", p=128)
                kraw = qstage.tile([128, TT], BF16, tag="qs")
                vst = vstage.tile([128, TT], BF16, tag="vst")
                nc.sync.dma_start(kraw, kvr[:, 0, :])
                nc.sync.dma_start(vst, kvr[:, 1, :])
                dstk = kp.tile([128, TT], BF16, tag="kt")
                k_tiles[ii] = dstk
                _rope(nc, rtmp, cos_sb[:, ts(ii, TT)],
                      sin_sb[:, ts(ii, TT)], kraw, dstk)
                v4 = vp.tile([128, 4, 128], BF16, tag="vt")
                v_tiles[ii] = v4
                for j4 in range(4):
                    tp = y_ps.tile([128, 128], BF16, tag="y")
                    nc.tensor.transpose(
                        tp, vst[:, j4 * 128:(j4 + 1) * 128], ident
                    )
                    nc.vector.tensor_copy(v4[:, j4, :], tp)

            # QKV runs three chunks ahead of attention: ~30us of PE work in
            # flight always covers the serialized ~12-20us collectives, and
            # x-chunk DMA demand stays spread over the whole kernel
            for ii in range(3):
                qkv(ii)

            for i in range(NT):
                if i + 3 < NT:
                    qkv(i + 3)
                receive(i)

                # ---- attention for t-chunk i, both heads ----
                # the previous chunk's out-projection pairs are interleaved
                # into this chunk's attention pair stream (one proj pair per
                # flushed attention pair) so attention matmuls fill the
                # proj-drain latency
                proj_queue = (
                    [(i - 1, op2) for op2 in range(NCT // 2)] if i > 0 else []
                )
                yts = y_chunks[i]
                ns = 4 * (i + 1)
                for h in range(2):
                    yp = y_ps.tile([128, TT], FP32, tag="y")
                    dp = aux_ps.tile([1, TT], FP32, tag="den")
                    den_started = [False]
                    held_prd = [None]

                    def den_mm(ap, off, is_last):
                        nc.tensor.matmul(
                            dp[:, off:], ones_sb, ap,
                            start=(not den_started[0]), stop=is_last,
                            skip_group_check=True,
                        )
                        den_started[0] = True

                    def flush(m, offs, p2, den_ap, diag):
                        # AV + den for pair m; emitted one pair late so the
                        # ACT exp latency is hidden behind the next pair's
                        # score matmuls. den_ap is the QUAD sum (4 s-tiles,
                        # second-level DVE pair-add) carried by every odd
                        # full pair; even full pairs carry no den work.
                        j0 = 2 * m
                        for h2 in range(2):
                            j = j0 + h2
                            off = offs[h2]
                            nc.tensor.matmul(
                                yp[:, off:], v_tiles[j // 4][:, j % 4, :],
                                p2[:, h2, off:],
                                start=(j == 0), stop=(j == ns - 1),
                                skip_group_check=True,
                            )
                        if den_ap is not None:
                            den_mm(den_ap, 0, False)
                        elif diag:
                            for h2 in range(2):
                                j = j0 + h2
                                den_mm(p2[:, h2, offs[h2]:], offs[h2],
                                       j == ns - 1)

                    pend = []
                    for m in range(ns // 2):
                        j0 = 2 * m
                        diag = j0 >= 4 * i
                        s2t = sp2.tile([128, 2, TT], FP32, tag="s2")
                        p2 = pp.tile([128, 2, TT], BF16, tag="p")
                        offs = []
                        for h2 in range(2):
                            j = j0 + h2
                            off = (j % 4) * 128 if j >= 4 * i else 0
                            offs.append(off)
                            nc.tensor.matmul(
                                s2t[:, h2, off:],
                                k_tiles[j // 4][:, (j % 4) * 128:(j % 4 + 1) * 128],
                                q_tiles[h][i][:, off:],
                                start=True,
                                stop=True,
                            )
                        den_ap = None
                        if not diag:
                            # one ACT instruction for both halves (1024 cols)
                            nc.scalar.activation(p2, s2t, AF.Exp, scale=SCALE)
                            prd = prp.tile([128, TT], BF16, tag="pr")
                            nc.vector.tensor_add(prd, p2[:, 0, :], p2[:, 1, :])
                            # second-level pair-add: one den matmul per QUAD
                            # of s-tiles (full pairs per chunk are even, so
                            # quads always close)
                            if held_prd[0] is None:
                                held_prd[0] = prd
                            else:
                                quad = prp.tile([128, TT], BF16, tag="pr")
                                nc.vector.tensor_add(quad, held_prd[0], prd)
                                held_prd[0] = None
                                den_ap = quad
                        else:
                            for h2 in range(2):
                                j = j0 + h2
                                off = offs[h2]
                                nv = TT - off
                                nc.scalar.activation(
                                    p2[:, h2, off:], s2t[:, h2, off:],
                                    AF.Exp, scale=SCALE,
                                )
                                # zero entries with s > t inside the aligned
                                # 128-wide triangle at the start of the
                                # slice: keep iff y - p >= 0
                                nc.gpsimd.affine_select(
                                    out=p2[:, h2, off:],
                                    in_=p2[:, h2, off:],
                                    compare_op=ALU.is_ge,
                                    fill=0.0,
                                    base=0,
                                    pattern=[[1, nv]],
                                    channel_multiplier=-1,
                                )
                        pend.append((m, offs, p2, den_ap, diag))
                        if len(pend) > 2:
                            flush(*pend.pop(0))
                            if proj_queue:
                                proj_pair(*proj_queue.pop(0))
                    for pe in pend:
                        flush(*pe)
                        if proj_queue:
                            proj_pair(*proj_queue.pop(0))

                    # softmax denominator -> broadcast to 128 rows via a tiny
                    # K=1 fp32r matmul on PE, then fast reciprocal
                    drow = rbp.tile([1, TT], F32R, tag="drow")
                    nc.vector.tensor_copy(drow, dp)
                    bc = aux_ps.tile([128, TT], FP32, tag="den")
                    nc.tensor.matmul(bc, ones_col, drow, start=True, stop=True)
                    rb = rbp.tile([128, TT], FP32, tag="rb")
                    nc.vector.reciprocal_approx_fast(out=rb, in_=bc)
                    yt = ysb.tile([128, TT], BF16, tag="yt")
                    nc.vector.tensor_mul(yt, yp, rb)
                    yts.append(yt)

                for pq in proj_queue:
                    proj_pair(*pq)

            emit_proj(NT - 1)

    nc.compile()
    return nc


def _prep_inputs(x, cos, sin, W_attn, W_proj):
    bf = ml_dtypes.bfloat16
    x = np.asarray(x, dtype=np.float32)
    cos = np.asarray(cos, dtype=np.float32)
    sin = np.asarray(sin, dtype=np.float32)
    W_attn = np.asarray(W_attn, dtype=np.float32)
    W_proj = np.asarray(W_proj, dtype=np.float32)

    xT = np.ascontiguousarray(x.reshape(T, C).T).astype(bf)
    cos2 = np.ascontiguousarray(np.concatenate([cos.T, cos.T], axis=0)).astype(bf)
    # partition-swapped + sign-folded: rows 0:64 = +sin (multiplies x1 into
    # the upper output half), rows 64:128 = -sin (multiplies x2 into the
    # lower half); see _rope
    sin2 = np.ascontiguousarray(
        np.concatenate([sin.T, -sin.T], axis=0)
    ).astype(bf)

    in_maps = []
    for core in range(8):
        g = core // 2
        qoff = g * 768 + (core % 2) * 256
        # even core of a pair computes the group's K rows, odd computes V;
        # the pair AllGathers so both end up with [k; v]
        kv = (
            W_attn[g * 768 + 512:g * 768 + 640]
            if core % 2 == 0
            else W_attn[g * 768 + 640:g * 768 + 768]
        )
        rows = np.concatenate([W_attn[qoff:qoff + 256], kv], axis=0)
        waT = np.ascontiguousarray(rows.T).astype(bf)
        h0 = g * 4 + (core % 2) * 2
        wpT = np.ascontiguousarray(W_proj[:, h0 * 128:h0 * 128 + 256].T).astype(bf)
        in_maps.append(
            {"xT": xT, "waT": waT, "wpT": wpT, "cos2": cos2, "sin2": sin2}
        )
    return in_maps


def kernel(x, cos, sin, W_attn, W_proj, _trace=False, _trace_cores=None):
    if "nc" not in _cache:
        _cache["nc"] = build()
    nc = _cache["nc"]
    in_maps = _prep_inputs(x, cos, sin, W_attn, W_proj)
    kwargs = {}
    if _trace:
        install_ntff_hook_shim()
        kwargs = dict(trace=True, trace_cores=_trace_cores or [0])
    res = run_bass_kernel_spmd(nc, in_maps, core_ids=list(range(8)), **kwargs)
    acc = np.zeros((C, T), dtype=np.float32)
    for r in res.results:
        acc += np.asarray(r["outT"], dtype=np.float32)
    out = np.ascontiguousarray(acc.T).reshape(1, T, C)
    _cache["last_results"] = res
    return out
